# revision 35
# baseline (speedup 1.0000x reference)
"""Trainium2 Bass kernel v5.3 for channel attention (XCA-style), 8 NeuronCores.

Data-parallel over batch B=8 (1 item/core, no collectives).

Changes vs v3 (316us baseline -> ~241us):
- Post-gram normalization: q/k are transposed RAW (no khat scaling pass);
  the gram G = Q K^T is computed unnormalized and the norms are applied at
  exp time - the q row-scale (x temperature) folds into a pre-transpose
  DVE scale of the logits, the k column-scale becomes a per-partition ACT
  scale after a PE transpose. Takes rinv off the k->gram critical path.
- Softmax row-sums via a ones-column matmul on the transposed exp; rs folds
  into the transpose-back evacuation scale.
- attn@v+proj fusion for chunks 0,2,3: WpA_g^T = (rs*A_g)^T Wp_g via one
  N=512 matmul per chunk, then y = sum_g WpA_g^T^T vdw_g runs directly
  against the DVE-depthwise'd v. Eliminates those chunks' attn@v stage and
  its PSUM evacuation. Chunk 1 keeps the fused 9-tap attn@v on the PE
  (fills the tail with PE work; its og aliases the dead xb slot 1, and the
  projection contracts chunks in order 0,2,1,3 so late work is needed last).
- Both q and k norm squares from the depthwise PSUM; one batched [128,8]
  DVE quake-rsqrt chain for all norms.
- q depthwise evacuation on the DVE (2x_1P PSUM read), k on ACT (split
  load); edge-wrap corrections merged to 64-row ops (6/chunk).
- keepwarm matmuls between tail attention chunks against HAM re-throttle.

Hard-won scheduling rules (measured, do not regress):
- Nothing on the q/k->gram critical path (corrections, scales) may queue
  behind multi-us v-depthwise taps in the DVE FIFO; attention chains
  inside phase 1 stall the PE via PSUM-slot holds and DVE FIFO ordering
  (tried twice: +45us and +70us).
- Interleaving pw/dw across chunks, finer tap slicing, running the fused
  attn@v first in the tail, and denser keepwarms all measured neutral to
  worse (242-250us); the tail is DVE/chain-latency-bound, not HAM-bound.

Kept from v3: fp8-e4m3 q/k path with DoubleRow everywhere, pitch-64 guarded
depthwise layout, fp8 weights prescaled x16, DVE v-depthwise with GpSimd
assist, PSUM in [128,1024] 2-bank groups.
"""

import os
import sys

import numpy as np

for _p in ("/opt/trn_rl_repo", "/root/.axon_site/_ro/trn_rl_repo"):
    if os.path.isdir(_p) and _p not in sys.path:
        sys.path.insert(0, _p)

import ml_dtypes

B, C, HH, WW = 8, 512, 64, 64
HEADS, D = 8, 64
HW = HH * WW          # 4096
G = C // 128          # 4 chunks of 128 channels (2 heads each)
NBK = 512             # one PSUM bank of fp32
PP = WW + 2           # bordered pad pitch for v (66)
NG = 66 * 64 + 2      # pitch-64 guarded flat size for q/k (4226)
WSCALE = 16.0         # fp8 weight prescale (undone at pw evacuation)
QTR = 1024            # 2-bank PSUM group width

# tap index t = dy*3+dx; flat stream offset for bank nb: 1 + nb*512 + TOFF[t]
TOFF = [dy * 64 + dx - 1 for dy in range(3) for dx in range(3)]
# DoubleRow tap pairs (a, b): dim1 stride must be >= 2 (d=1 wedges the device)
DW_PAIRS = [(0, 2), (1, 4), (3, 6), (5, 8), (None, 7)]
FUSED = (1,)

_CACHE = {}


def _build():
    from contextlib import ExitStack

    import concourse.tile as tile
    from concourse import bacc, mybir
    from concourse.ap import AP

    f32 = mybir.dt.float32
    bf16 = mybir.dt.bfloat16
    f8 = mybir.dt.float8e4
    i32 = mybir.dt.int32
    AO = mybir.AluOpType
    AF = mybir.ActivationFunctionType
    PM = mybir.MatmulPerfMode

    nc = bacc.Bacc()

    x8_ext = nc.declare_dram_parameter("x8", [128, G, HW], f8, isOutput=False)
    xb_ext = nc.declare_dram_parameter("xb", [128, G, HW], bf16, isOutput=False)
    w8_ext = {t: nc.declare_dram_parameter(f"w8{t}", [128, G, C], f8,
                                           isOutput=False) for t in "qk"}
    wv_ext = nc.declare_dram_parameter("wv", [128, G, C], bf16, isOutput=False)
    wp_ext = nc.declare_dram_parameter("wp", [128, G, C], bf16, isOutput=False)
    dg_ext = {t: nc.declare_dram_parameter(f"dg{t}", [128, G, 5, 2, 128], f8,
                                           isOutput=False) for t in "qk"}
    ndw_ext = {t: nc.declare_dram_parameter(f"ndw{t}", [128, G, 9], f32,
                                            isOutput=False) for t in "qk"}
    dwv_ext = nc.declare_dram_parameter("dwv", [128, G, 9], f32, isOutput=False)
    tsc_ext = nc.declare_dram_parameter("tsc", [128, G], f32, isOutput=False)
    id_ext = nc.declare_dram_parameter("ident", [128, 128], bf16, isOutput=False)
    out_ext = nc.declare_dram_parameter("out", [C, HW], bf16, isOutput=True)

    with ExitStack() as ctx:
        tc = ctx.enter_context(tile.TileContext(nc))
        sb = ctx.enter_context(tc.tile_pool(name="sb", bufs=1))
        ps = ctx.enter_context(tc.tile_pool(name="ps", bufs=1, space="PSUM"))

        def win(tile_, off, dims):
            a = tile_[:, off:off + 1]
            return AP(a.tensor, a.offset, [list(a.ap[0])] + dims)

        # ---- PE warm-up: ramp the pstate during the initial DMA wait ----
        warm = sb.tile([128, 128], bf16, name="warm", tag="warm")
        nc.vector.memset(warm, 0.0)
        ones = sb.tile([128, 1], bf16, name="ones", tag="ones")
        nc.vector.memset(ones, 1.0)
        wp_ps = ps.tile([128, NBK], f32, name="warmps", tag="ps2", bufs=4)
        for i in range(24):
            nc.tensor.matmul(wp_ps[:, 0:128], lhsT=warm, rhs=warm,
                             start=(i == 0), stop=(i == 23))

        # ---- persistent loads (q-path first so pw can start ASAP) -------
        w8q = sb.tile([128, G, C], f8, name="w8q", tag="w8q")
        nc.sync.dma_start(out=w8q, in_=w8_ext["q"][:, :, :])
        x8 = sb.tile([128, G, HW], f8, name="x8", tag="x8")
        nc.sync.dma_start(out=x8[:, 0:2, :], in_=x8_ext[:, 0:2, :])
        nc.sync.dma_start(out=x8[:, 2:4, :], in_=x8_ext[:, 2:4, :])
        dgq = sb.tile([128, G, 5, 2, 128], f8, name="dgq", tag="dgq")
        nc.sync.dma_start(out=dgq, in_=dg_ext["q"][:, :, :, :, :])
        ndwq = sb.tile([128, G, 9], f32, name="ndwq", tag="ndwq")
        nc.sync.dma_start(out=ndwq, in_=ndw_ext["q"][:, :, :])
        tsc = sb.tile([128, G], f32, name="tsc", tag="tsc")
        nc.sync.dma_start(out=tsc, in_=tsc_ext[:, :])
        w8k = sb.tile([128, G, C], f8, name="w8k", tag="w8k")
        nc.sync.dma_start(out=w8k, in_=w8_ext["k"][:, :, :])
        dgk = sb.tile([128, G, 5, 2, 128], f8, name="dgk", tag="dgk")
        nc.sync.dma_start(out=dgk, in_=dg_ext["k"][:, :, :, :, :])
        ndwk = sb.tile([128, G, 9], f32, name="ndwk", tag="ndwk")
        nc.sync.dma_start(out=ndwk, in_=ndw_ext["k"][:, :, :])
        xb = sb.tile([128, G, HW], bf16, name="xb", tag="xb")
        nc.sync.dma_start(out=xb, in_=xb_ext[:, :, :])
        wv = sb.tile([128, G, C], bf16, name="wv", tag="wvp")
        nc.sync.dma_start(out=wv, in_=wv_ext[:, :, :])
        dwv = sb.tile([128, G, 9], f32, name="dwv", tag="dwv")
        nc.sync.dma_start(out=dwv, in_=dwv_ext[:, :, :])
        ident = sb.tile([128, 128], bf16, name="ident", tag="ident")
        nc.sync.dma_start(out=ident, in_=id_ext[:, :])

        w8 = {"q": w8q, "k": w8k}
        dg = {"q": dgq, "k": dgk}
        ndw = {"q": ndwq, "k": ndwk}

        # persistent per-chunk results
        vpad = [None] * G
        qT = [None] * G
        kT = [None] * G
        vdw = {}
        wpa = [None] * G
        gsb = [None] * G
        ssall = sb.tile([128, 2 * G], f32, name="ssall", tag="ssall")

        # ---- q/k pointwise + depthwise, all fp8 DoubleRow ----------------
        def qk_chunk(t, g):
            """fp8 pw into guarded qg, fp8-DR depthwise into bf16 acc,
            PSUM norm squares, corrections, raw transpose."""
            qg = sb.tile([128, NG], f8, name=f"qg_{t}{g}", tag="qg", bufs=2)
            # zero guards: front elem + row0 [0..64], row65 + tail [4161..]
            nc.gpsimd.memset(qg[:, 0:65], 0.0)
            nc.gpsimd.memset(qg[:, 1 + 64 * 65:NG], 0.0)
            for q4 in range(4):                     # 2-bank quarters
                pw = ps.tile([128, QTR], f32, name=f"pw_{t}{g}{q4}",
                             tag="ps2", bufs=4)
                for j in range(2):                  # k-chunk pairs
                    lhsT = w8[t][:, 2 * j:2 * j + 2, g * 128:(g + 1) * 128]
                    for nb in range(2):
                        nc.tensor.matmul(
                            pw[:, nb * NBK:(nb + 1) * NBK], lhsT=lhsT,
                            rhs=x8[:, 2 * j:2 * j + 2,
                                   (2 * q4 + nb) * NBK:(2 * q4 + nb + 1) * NBK],
                            start=(j == 0), stop=(j == 1),
                            perf_mode=PM.DoubleRow)
                # rows 16*q4+1 .. 16*q4+16 of the guarded tile, contiguous
                nc.scalar.activation(
                    out=qg[:, 1 + 64 * (16 * q4 + 1):1 + 64 * (16 * q4 + 17)],
                    in_=pw, func=AF.Copy, scale=1.0 / WSCALE)
            acc = sb.tile([128, HW], bf16, name=f"acc_{t}{g}", tag="acc",
                          bufs=2)
            ssp = sb.tile([128, 4], f32, name=f"ssp_{t}{g}", tag="nrm_ssp",
                          bufs=2)
            for q4 in range(4):
                dwp = ps.tile([128, QTR], f32, name=f"dw_{t}{g}{q4}",
                              tag="ps2", bufs=4)
                for i, (ta, tb) in enumerate(DW_PAIRS):
                    offa = TOFF[ta] if ta is not None else TOFF[tb] - 2
                    d = TOFF[tb] - offa
                    lhsT = dg[t][:, g, i, :, :]
                    for nb in range(2):
                        base = 1 + (2 * q4 + nb) * NBK + offa
                        nc.tensor.matmul(
                            dwp[:, nb * NBK:(nb + 1) * NBK], lhsT=lhsT,
                            rhs=win(qg, base, [[d, 2], [1, NBK]]),
                            start=(i == 0), stop=(i == len(DW_PAIRS) - 1),
                            perf_mode=PM.DoubleRow)
                if t == "q":
                    nc.vector.tensor_copy(acc[:, q4 * QTR:(q4 + 1) * QTR],
                                          dwp)
                else:
                    nc.scalar.copy(acc[:, q4 * QTR:(q4 + 1) * QTR], dwp)
                nc.scalar.activation(out=dwp, in_=dwp, func=AF.Square,
                                     accum_out=ssp[:, q4:q4 + 1])
            si = (0 if t == "q" else G) + g
            nc.vector.tensor_tensor(out=ssp[:, 0:1], in0=ssp[:, 0:1],
                                    in1=ssp[:, 1:2], op=AO.add)
            nc.vector.tensor_tensor(out=ssp[:, 2:3], in0=ssp[:, 2:3],
                                    in1=ssp[:, 3:4], op=AO.add)
            nc.vector.tensor_tensor(out=ssall[:, si:si + 1], in0=ssp[:, 0:1],
                                    in1=ssp[:, 2:3], op=AO.add)
            # edge-wrap fixes on cols 0/63, negated taps
            acc3 = acc.rearrange("p (h w) -> p h w", w=WW)
            for dy in range(3):
                nc.vector.scalar_tensor_tensor(
                    out=acc3[:, :, 0:1],
                    in0=win(qg, 64 * dy, [[64, 64], [1, 1]]),
                    scalar=ndw[t][:, g, 3 * dy:3 * dy + 1],
                    in1=acc3[:, :, 0:1], op0=AO.mult, op1=AO.add)
                nc.vector.scalar_tensor_tensor(
                    out=acc3[:, :, 63:64],
                    in0=win(qg, 1 + 64 * (dy + 1), [[64, 64], [1, 1]]),
                    scalar=ndw[t][:, g, 3 * dy + 2:3 * dy + 3],
                    in1=acc3[:, :, 63:64], op0=AO.mult, op1=AO.add)
            tt = sb.tile([128, HW], bf16, name=f"{t}T{g}", tag=f"{t}T",
                         bufs=2)
            dst3 = tt.rearrange("p (a c) -> p a c", c=128)
            for h in range(2):
                nc.sync.dma_start(out=dst3[:, 16 * h:16 * (h + 1), :],
                                  in_=acc[:, 2048 * h:2048 * (h + 1)],
                                  transpose=True)
            (qT if t == "q" else kT)[g] = tt

        def gram(g):
            gp = ps.tile([128, QTR], f32, name=f"gram{g}", tag="ps2", bufs=4)
            for nck in range(32):
                nc.tensor.matmul(
                    gp[:, 0:128],
                    lhsT=qT[g][:, nck * 128:(nck + 1) * 128],
                    rhs=kT[g][:, nck * 128:(nck + 1) * 128],
                    start=(nck == 0), stop=(nck == 31))
            gs = sb.tile([128, 128], f32, name=f"gsb{g}", tag="gsb", bufs=4)
            nc.scalar.copy(gs, gp[:, 0:128])
            gsb[g] = gs

        # ---- v pointwise (bf16) into bordered 66-pitch pad ---------------
        def v_pw(g):
            pad = sb.tile([128, PP, PP], bf16, name=f"vpad{g}", tag="vpadp",
                          bufs=3)
            vpad[g] = pad
            padf = pad.rearrange("p h w -> p (h w)")
            nc.gpsimd.memset(padf[:, 0:PP], 0.0)
            nc.gpsimd.memset(padf[:, (PP - 1) * PP:PP * PP], 0.0)
            nc.gpsimd.memset(pad[:, 1:PP - 1, 0:1], 0.0)
            nc.gpsimd.memset(pad[:, 1:PP - 1, PP - 1:PP], 0.0)
            for q4 in range(4):
                pw = ps.tile([128, QTR], f32, name=f"vpw{g}{q4}",
                             tag="ps2", bufs=4)
                for k in range(G):
                    lhsT = wv[:, k, g * 128:(g + 1) * 128]
                    for nb in range(2):
                        nc.tensor.matmul(
                            pw[:, nb * NBK:(nb + 1) * NBK], lhsT=lhsT,
                            rhs=xb[:, k,
                                   (2 * q4 + nb) * NBK:(2 * q4 + nb + 1) * NBK],
                            start=(k == 0), stop=(k == G - 1))
                nc.scalar.copy(
                    pad[:, 1 + q4 * 16:1 + (q4 + 1) * 16, 1:WW + 1],
                    pw.rearrange("p (h w) -> p h w", w=WW))

        vdw3 = {}

        # ---- explicit v depthwise on DVE (4x mult + 2x add chain) --------
        def v_dw_dve(g, taps, seed=4):
            pad = vpad[g]
            if g not in vdw:
                if g in (3,):
                    # alias into the dead xb slot g (xb is done after v_pw(3))
                    av = xb[:, g, :]
                    vdw[g] = av
                    vdw3[g] = AP(av.tensor, av.offset,
                                 [list(av.ap[0]), [WW, HH], [1, WW]])
                else:
                    acc = sb.tile([128, HW], bf16, name=f"vdw{g}",
                                  tag=f"vdw{g}")
                    vdw[g] = acc
                    vdw3[g] = acc.rearrange("p (h w) -> p h w", w=WW)
                acc3 = vdw3[g]
                sy, sx = seed // 3, seed % 3
                nc.vector.tensor_scalar(
                    out=acc3[:, :, :], in0=pad[:, sy:sy + HH, sx:sx + WW],
                    scalar1=dwv[:, g, seed:seed + 1], scalar2=None,
                    op0=AO.mult)
            acc3 = vdw3[g]
            tmp = sb.tile([128, HH, WW], bf16, name=f"vt{g}", tag="vtmp",
                          bufs=1)
            for tap in taps:
                dy, dx = tap // 3, tap % 3
                nc.vector.tensor_scalar(
                    out=tmp, in0=pad[:, dy:dy + HH, dx:dx + WW],
                    scalar1=dwv[:, g, tap:tap + 1], scalar2=None, op0=AO.mult)
                nc.vector.tensor_tensor(out=acc3, in0=acc3, in1=tmp,
                                        op=AO.add)

        def v_dw2_gps(g, tap):
            # tap contribution in halves: DVE scaled-mult, gpsimd add
            pad = vpad[g]
            dy, dx = tap // 3, tap % 3
            for h in range(4):
                tmp = sb.tile([128, 16, WW], bf16, name=f"v2t{tap}{h}",
                              tag="vtmp2", bufs=1)
                nc.vector.tensor_scalar(
                    out=tmp, in0=pad[:, dy + 16 * h:dy + 16 * (h + 1),
                                     dx:dx + WW],
                    scalar1=dwv[:, g, tap:tap + 1], scalar2=None, op0=AO.mult)
                a3v = vdw3[g]
                nc.gpsimd.tensor_tensor(
                    out=a3v[:, 16 * h:16 * (h + 1), :],
                    in0=a3v[:, 16 * h:16 * (h + 1), :], in1=tmp, op=AO.add)

        # batched rsqrt of all 8 norms: quake bit-hack + 2 Newton steps
        def rinv_all():
            NSS = 2 * G
            sh = sb.tile([128, NSS], i32, name="nrm_sh", tag="nrm_sh")
            nc.vector.tensor_scalar(out=sh, in0=ssall.bitcast(i32), scalar1=1,
                                    scalar2=None, op0=AO.logical_shift_right)
            y0i = sb.tile([128, NSS], i32, name="nrm_y0", tag="nrm_y0")
            eng = nc.vector
            eng.add_instruction(mybir.InstTensorScalarPtr(
                name=nc.get_next_instruction_name(),
                op0=AO.subtract, reverse0=True,
                ins=[eng.lower_ap(sh[:, :]),
                     mybir.ImmediateValue(dtype=i32, value=0x5f3759df)],
                outs=[eng.lower_ap(y0i[:, :])]))
            rinv = sb.tile([128, NSS], f32, name="rinv", tag="rinv")
            nc.vector.tensor_copy(rinv, y0i.bitcast(f32))
            tn = sb.tile([128, NSS], f32, name="nrm_tn", tag="nrm_tn")
            for _ in range(2):
                nc.vector.tensor_tensor(out=tn, in0=rinv, in1=rinv, op=AO.mult)
                nc.vector.tensor_tensor(out=tn, in0=tn, in1=ssall, op=AO.mult)
                nc.vector.tensor_scalar(out=tn, in0=tn, scalar1=-0.5,
                                        scalar2=1.5, op0=AO.mult, op1=AO.add)
                nc.vector.tensor_tensor(out=rinv, in0=rinv, in1=tn, op=AO.mult)
            ts_scale = sb.tile([128, G], f32, name="tss", tag="tss")
            nc.vector.tensor_tensor(out=ts_scale, in0=tsc,
                                    in1=rinv[:, 0:G], op=AO.mult)
            return ts_scale, rinv

        # ---- attention for one chunk (2 heads), fused into proj lhsT -----
        def attn_chunk(g, ts_scale, rinv, wp):
            # logits = gram * (T*rq[d]) -> transpose -> exp(rk[e] * .)
            tmp = sb.tile([128, 128], bf16, name=f"tmp{g}", tag="atmp",
                          bufs=2)
            nc.vector.tensor_scalar(out=tmp, in0=gsb[g],
                                    scalar1=ts_scale[:, g:g + 1], scalar2=None,
                                    op0=AO.mult)
            ap_ = ps.tile([128, QTR], f32, name=f"attn{g}", tag="ps2", bufs=4)
            nc.tensor.matmul(ap_[:, 0:128], lhsT=tmp, rhs=ident,
                             start=True, stop=True)
            aexpT = sb.tile([128, 128], bf16, name=f"aexpT{g}", tag="aexpT",
                            bufs=2)
            nc.vector.memset(aexpT, 0.0)
            for blk in (0, 64):
                nc.scalar.activation(
                    out=aexpT[blk:blk + 64, blk:blk + 64],
                    in_=ap_[blk:blk + 64, blk:blk + 64],
                    func=AF.Exp, scale=rinv[blk:blk + 64, G + g:G + g + 1])
            nc.tensor.matmul(ap_[:, 512:513], lhsT=aexpT, rhs=ones,
                             start=True, stop=True)
            rs = sb.tile([128, 1], f32, name=f"rs{g}", tag="rsum", bufs=2)
            nc.vector.reciprocal(rs, ap_[:, 512:513])
            if g in FUSED:
                # fused attn@v: out = sum_tap (aexpT . dwv_tap) @ shifted v
                og = xb[:, g, :]
                a3 = []
                for tap in range(9):
                    a3t = sb.tile([128, 128], bf16, name=f"a3_{g}{tap}",
                                  tag=f"a3_{tap}", bufs=1)
                    nc.vector.tensor_scalar(out=a3t, in0=aexpT,
                                            scalar1=dwv[:, g, tap:tap + 1],
                                            scalar2=None, op0=AO.mult)
                    a3.append(a3t)
                for q4 in range(4):
                    vo = ps.tile([128, QTR], f32, name=f"fo{g}{q4}",
                                 tag="ps2", bufs=4)
                    for tap in range(9):
                        dy, dx = tap // 3, tap % 3
                        for nb in range(2):
                            r0 = (2 * q4 + nb) * 8
                            nc.tensor.matmul(
                                vo[:, nb * NBK:(nb + 1) * NBK], lhsT=a3[tap],
                                rhs=vpad[g][:, dy + r0:dy + r0 + 8,
                                            dx:dx + WW],
                                start=(tap == 0), stop=(tap == 8))
                    nc.scalar.activation(out=og[:, q4 * QTR:(q4 + 1) * QTR],
                                         in_=vo, func=AF.Copy, scale=rs)
                vdw[g] = og
                return
            # transpose back; fold rs into the evacuation scale
            nc.tensor.matmul(ap_[:, 256:384], lhsT=aexpT, rhs=ident,
                             start=True, stop=True)
            aexp = sb.tile([128, 128], bf16, name=f"aexp{g}", tag="aexp",
                           bufs=2)
            nc.scalar.activation(out=aexp, in_=ap_[:, 256:384], func=AF.Copy,
                                 scale=rs)
            # WpA_g^T = (rs*A_g)^T @ Wp_g  [e, o] - one N=512 matmul
            nc.tensor.matmul(ap_[:, 512:1024], lhsT=aexp, rhs=wp[:, g, :],
                             start=True, stop=True)
            wa = sb.tile([128, C], bf16, name=f"wpa{g}", tag="wpa", bufs=4)
            nc.scalar.copy(wa, ap_[:, 512:1024])
            wpa[g] = wa

        # ================= main schedule =================================
        kw_idx = [0]

        def keepwarm(n=4):
            kw_idx[0] += 1
            kw_ps = ps.tile([128, QTR], f32, name=f"kwps{kw_idx[0]}",
                            tag="ps2", bufs=4)
            for i in range(n):
                nc.tensor.matmul(kw_ps[:, 0:128], lhsT=warm, rhs=warm,
                                 start=(i == 0), stop=(i == n - 1))

        qk_chunk("q", 0)
        qk_chunk("k", 0)
        v_pw(0)
        gram(0)
        qk_chunk("q", 1)
        v_dw_dve(0, (0, 1, 2))
        qk_chunk("k", 1)
        v_dw_dve(0, (3, 5))
        v_pw(1)
        gram(1)
        v_dw_dve(0, (6,))
        v_dw2_gps(0, 7)
        qk_chunk("q", 2)
        v_dw_dve(0, (8,))
        qk_chunk("k", 2)
        v_pw(2)
        gram(2)
        v_pw(3)
        v_dw_dve(2, (0, 1, 2))
        qk_chunk("q", 3)
        v_dw_dve(2, (3, 5))
        v_dw2_gps(2, 7)
        v_dw_dve(3, (0, 1))
        v_dw2_gps(3, 7)
        qk_chunk("k", 3)
        v_dw_dve(3, (2, 3))
        gram(3)
        v_dw_dve(2, (6, 8))
        # wv is dead now: load proj weights into its slot
        wp = sb.tile([128, G, C], bf16, name="wp", tag="wvp")
        nc.sync.dma_start(out=wp, in_=wp_ext[:, :, :])
        ts_scale, rinv = rinv_all()
        attn_chunk(0, ts_scale, rinv, wp)
        v_dw_dve(3, (5, 6))
        keepwarm()
        attn_chunk(2, ts_scale, rinv, wp)
        v_dw_dve(3, (8,))
        keepwarm()
        attn_chunk(3, ts_scale, rinv, wp)
        attn_chunk(1, ts_scale, rinv, wp)

        # ================= fused projection + store ======================
        # y[:, strip] = sum_g WpA_g^T^T @ vdw_g[:, strip]
        for q4 in range(4):
            for m in range(G):
                yp = ps.tile([128, QTR], f32, name=f"yp{m}{q4}",
                             tag="ps2", bufs=4)
                for gi, g in enumerate((0, 2, 3, 1)):
                    lhsT = (wp[:, g, m * 128:(m + 1) * 128] if g in FUSED
                            else wpa[g][:, m * 128:(m + 1) * 128])
                    for nb in range(2):
                        nc.tensor.matmul(
                            yp[:, nb * NBK:(nb + 1) * NBK], lhsT=lhsT,
                            rhs=vdw[g][:, (2 * q4 + nb) * NBK:
                                       (2 * q4 + nb + 1) * NBK],
                            start=(gi == 0), stop=(gi == G - 1))
                yt = sb.tile([128, QTR], bf16, name=f"yt{m}{q4}",
                             tag="ysb", bufs=2)
                nc.scalar.copy(yt, yp)
                nc.sync.dma_start(
                    out=out_ext[m * 128:(m + 1) * 128,
                                q4 * QTR:(q4 + 1) * QTR],
                    in_=yt)

    nc.compile()
    return nc


def prep_inputs(x, w_q, w_k, w_v, dw_q, dw_k, dw_v, w_proj, temperature):
    bf = ml_dtypes.bfloat16
    f8 = ml_dtypes.float8_e4m3
    xf = np.ascontiguousarray(np.asarray(x, np.float32)).reshape(B, C, HW)
    # [C, HW] -> [128, G, HW]
    xg = xf.reshape(B, G, 128, HW).transpose(0, 2, 1, 3)

    def wprep(w, dtype, scale=1.0):
        # w [O, I] -> lhsT layout [128 (i in chunk), G (i chunk), O]
        wt = (np.asarray(w, np.float32).T * scale).reshape(G, 128, C)
        return np.ascontiguousarray(wt.transpose(1, 0, 2)).astype(dtype)

    def dwprep(dw):
        # [C,1,3,3] -> [128, G, 9]
        d = np.asarray(dw, np.float32).reshape(G, 128, 9)
        return np.ascontiguousarray(d.transpose(1, 0, 2))

    def dgprep(dw9):
        # dw9 [128, G, 9] f32 (already fp8-rounded) -> diag pairs
        dgt = np.zeros((128, G, 5, 2, 128), np.float32)
        r = np.arange(128)
        for i, (ta, tb) in enumerate(DW_PAIRS):
            if ta is not None:
                dgt[r, :, i, 0, r] = dw9[r, :, ta]
            dgt[r, :, i, 1, r] = dw9[r, :, tb]
        return dgt.astype(f8)

    dwq9 = dwprep(dw_q).astype(f8).astype(np.float32)
    dwk9 = dwprep(dw_k).astype(f8).astype(np.float32)
    base = {
        "w8q": wprep(w_q, f8, WSCALE),
        "w8k": wprep(w_k, f8, WSCALE),
        "wv": wprep(w_v, bf),
        "wp": wprep(w_proj, bf),
        "dgq": dgprep(dwq9),
        "dgk": dgprep(dwk9),
        "ndwq": np.ascontiguousarray(-dwq9),
        "ndwk": np.ascontiguousarray(-dwk9),
        "dwv": dwprep(dw_v),
        "tsc": np.ascontiguousarray(np.repeat(
            np.asarray(temperature, np.float32).reshape(HEADS), D)
            .reshape(G, 128).T),
        "ident": np.eye(128, dtype=bf),
    }
    in_maps = []
    for b in range(B):
        m = dict(base)
        m["x8"] = np.ascontiguousarray(xg[b]).astype(f8)
        m["xb"] = np.ascontiguousarray(xg[b]).astype(bf)
        in_maps.append(m)
    return in_maps


def run(trace=False, **inputs):
    from concourse.bass_utils import run_bass_kernel_spmd

    if "nc" not in _CACHE:
        _CACHE["nc"] = _build()
    nc = _CACHE["nc"]
    in_maps = prep_inputs(**inputs)
    res = run_bass_kernel_spmd(nc, in_maps, core_ids=list(range(B)),
                               trace=trace)
    out = np.stack([np.asarray(res.results[b]["out"], np.float32)
                    for b in range(B)])
    return out.reshape(B, C, HH, WW), res


def kernel(**inputs):
    out, _ = run(trace=False, **inputs)
    return out


# revision 36
# speedup vs baseline: 1.0228x; 1.0228x over previous
"""Trainium2 Bass kernel v5.3 for channel attention (XCA-style), 8 NeuronCores.

Data-parallel over batch B=8 (1 item/core, no collectives).

Changes vs v3 (316us baseline -> ~241us):
- Post-gram normalization: q/k are transposed RAW (no khat scaling pass);
  the gram G = Q K^T is computed unnormalized and the norms are applied at
  exp time - the q row-scale (x temperature) folds into a pre-transpose
  DVE scale of the logits, the k column-scale becomes a per-partition ACT
  scale after a PE transpose. Takes rinv off the k->gram critical path.
- Softmax row-sums via a ones-column matmul on the transposed exp; rs folds
  into the transpose-back evacuation scale.
- attn@v+proj fusion for chunks 0,2,3: WpA_g^T = (rs*A_g)^T Wp_g via one
  N=512 matmul per chunk, then y = sum_g WpA_g^T^T vdw_g runs directly
  against the DVE-depthwise'd v. Chunk 1 keeps the fused 9-tap attn@v on
  the PE (fills the tail with PE work; its og aliases the dead xb slot 1,
  and the projection contracts chunks in order 0,2,1,3).
- Both q and k norm squares from the depthwise PSUM; one batched [128,8]
  DVE quake-rsqrt chain for all norms.
- q depthwise evacuation on the DVE (2x_1P PSUM read), k on ACT (split
  load); edge-wrap corrections merged to 64-row ops (6/chunk).
- keepwarm matmuls between tail attention chunks against HAM re-throttle.

Measured do-not-regress facts (all alternatives tried and worse):
- Phase 1 is 95.5% PE-busy (131.9us busy / 138.2us span) - optimal.
- Nothing on the q/k->gram critical path may queue behind multi-us
  v-depthwise taps in the DVE FIFO; in-phase-1 attention chains (+45/+70us),
  pw/dw interleave (+40), finer tap slices (+8), fused-attn1-first tail
  (+8), dense keepwarms (+5), hoisting v_pw(3)+c3 taps into phase 1 (+8)
  all regressed. The tail is cross-engine chain-latency-bound.

Kept from v3: fp8-e4m3 q/k path with DoubleRow everywhere, pitch-64 guarded
depthwise layout, fp8 weights prescaled x16, DVE v-depthwise with GpSimd
assist, PSUM in [128,1024] 2-bank groups.
"""

import os
import sys

import numpy as np

for _p in ("/opt/trn_rl_repo", "/root/.axon_site/_ro/trn_rl_repo"):
    if os.path.isdir(_p) and _p not in sys.path:
        sys.path.insert(0, _p)

import ml_dtypes

B, C, HH, WW = 8, 512, 64, 64
HEADS, D = 8, 64
HW = HH * WW          # 4096
G = C // 128          # 4 chunks of 128 channels (2 heads each)
NBK = 512             # one PSUM bank of fp32
PP = WW + 2           # bordered pad pitch for v (66)
NG = 66 * 64 + 2      # pitch-64 guarded flat size for q/k (4226)
WSCALE = 16.0         # fp8 weight prescale (undone at pw evacuation)
QTR = 1024            # 2-bank PSUM group width

# tap index t = dy*3+dx; flat stream offset for bank nb: 1 + nb*512 + TOFF[t]
TOFF = [dy * 64 + dx - 1 for dy in range(3) for dx in range(3)]
# DoubleRow tap pairs (a, b): dim1 stride must be >= 2 (d=1 wedges the device)
DW_PAIRS = [(0, 2), (1, 4), (3, 6), (5, 8), (None, 7)]
FUSED = (1,)

_CACHE = {}


def _build():
    from contextlib import ExitStack

    import concourse.tile as tile
    from concourse import bacc, mybir
    from concourse.ap import AP

    f32 = mybir.dt.float32
    bf16 = mybir.dt.bfloat16
    f8 = mybir.dt.float8e4
    i32 = mybir.dt.int32
    AO = mybir.AluOpType
    AF = mybir.ActivationFunctionType
    PM = mybir.MatmulPerfMode

    nc = bacc.Bacc()

    x8_ext = nc.declare_dram_parameter("x8", [128, G, HW], f8, isOutput=False)
    xb_ext = nc.declare_dram_parameter("xb", [128, G, HW], bf16, isOutput=False)
    w8_ext = {t: nc.declare_dram_parameter(f"w8{t}", [128, G, C], f8,
                                           isOutput=False) for t in "qk"}
    wv_ext = nc.declare_dram_parameter("wv", [128, G, C], bf16, isOutput=False)
    wp_ext = nc.declare_dram_parameter("wp", [128, G, C], bf16, isOutput=False)
    dg_ext = {t: nc.declare_dram_parameter(f"dg{t}", [128, G, 5, 2, 128], f8,
                                           isOutput=False) for t in "qk"}
    ndw_ext = {t: nc.declare_dram_parameter(f"ndw{t}", [128, G, 9], f32,
                                            isOutput=False) for t in "qk"}
    dwv_ext = nc.declare_dram_parameter("dwv", [128, G, 9], f32, isOutput=False)
    tsc_ext = nc.declare_dram_parameter("tsc", [128, G], f32, isOutput=False)
    id_ext = nc.declare_dram_parameter("ident", [128, 128], bf16, isOutput=False)
    out_ext = nc.declare_dram_parameter("out", [C, HW], bf16, isOutput=True)

    with ExitStack() as ctx:
        tc = ctx.enter_context(tile.TileContext(nc))
        sb = ctx.enter_context(tc.tile_pool(name="sb", bufs=1))
        ps = ctx.enter_context(tc.tile_pool(name="ps", bufs=1, space="PSUM"))

        def win(tile_, off, dims):
            a = tile_[:, off:off + 1]
            return AP(a.tensor, a.offset, [list(a.ap[0])] + dims)

        # ---- PE warm-up: ramp the pstate during the initial DMA wait ----
        warm = sb.tile([128, 128], bf16, name="warm", tag="warm")
        nc.vector.memset(warm, 0.0)
        ones = sb.tile([128, 1], bf16, name="ones", tag="ones")
        nc.vector.memset(ones, 1.0)
        wp_ps = ps.tile([128, NBK], f32, name="warmps", tag="ps2", bufs=4)
        for i in range(24):
            nc.tensor.matmul(wp_ps[:, 0:128], lhsT=warm, rhs=warm,
                             start=(i == 0), stop=(i == 23))

        # ---- persistent loads (q-path first so pw can start ASAP) -------
        w8q = sb.tile([128, G, C], f8, name="w8q", tag="w8q")
        nc.sync.dma_start(out=w8q, in_=w8_ext["q"][:, :, :])
        x8 = sb.tile([128, G, HW], f8, name="x8", tag="x8")
        nc.sync.dma_start(out=x8[:, 0:2, :], in_=x8_ext[:, 0:2, :])
        nc.sync.dma_start(out=x8[:, 2:4, :], in_=x8_ext[:, 2:4, :])
        dgq = sb.tile([128, G, 5, 2, 128], f8, name="dgq", tag="dgq")
        nc.sync.dma_start(out=dgq, in_=dg_ext["q"][:, :, :, :, :])
        ndwq = sb.tile([128, G, 9], f32, name="ndwq", tag="ndwq")
        nc.sync.dma_start(out=ndwq, in_=ndw_ext["q"][:, :, :])
        tsc = sb.tile([128, G], f32, name="tsc", tag="tsc")
        nc.sync.dma_start(out=tsc, in_=tsc_ext[:, :])
        w8k = sb.tile([128, G, C], f8, name="w8k", tag="w8k")
        nc.sync.dma_start(out=w8k, in_=w8_ext["k"][:, :, :])
        dgk = sb.tile([128, G, 5, 2, 128], f8, name="dgk", tag="dgk")
        nc.sync.dma_start(out=dgk, in_=dg_ext["k"][:, :, :, :, :])
        ndwk = sb.tile([128, G, 9], f32, name="ndwk", tag="ndwk")
        nc.sync.dma_start(out=ndwk, in_=ndw_ext["k"][:, :, :])
        xb = sb.tile([128, G, HW], bf16, name="xb", tag="xb")
        nc.sync.dma_start(out=xb, in_=xb_ext[:, :, :])
        wv = sb.tile([128, G, C], bf16, name="wv", tag="wvp")
        nc.sync.dma_start(out=wv, in_=wv_ext[:, :, :])
        dwv = sb.tile([128, G, 9], f32, name="dwv", tag="dwv")
        nc.sync.dma_start(out=dwv, in_=dwv_ext[:, :, :])
        ident = sb.tile([128, 128], bf16, name="ident", tag="ident")
        nc.sync.dma_start(out=ident, in_=id_ext[:, :])

        w8 = {"q": w8q, "k": w8k}
        dg = {"q": dgq, "k": dgk}
        ndw = {"q": ndwq, "k": ndwk}

        # persistent per-chunk results
        vpad = [None] * G
        qT = [None] * G
        kT = [None] * G
        vdw = {}
        wpa = [None] * G
        gsb = [None] * G
        ssall = sb.tile([128, 2 * G], f32, name="ssall", tag="ssall")

        # ---- q/k pointwise + depthwise, all fp8 DoubleRow ----------------
        def qk_chunk(t, g):
            """fp8 pw into guarded qg, fp8-DR depthwise into bf16 acc,
            PSUM norm squares, corrections, raw transpose."""
            qg = sb.tile([128, NG], f8, name=f"qg_{t}{g}", tag="qg", bufs=2)
            # zero guards: front elem + row0 [0..64], row65 + tail [4161..]
            nc.gpsimd.memset(qg[:, 0:65], 0.0)
            nc.gpsimd.memset(qg[:, 1 + 64 * 65:NG], 0.0)
            for q4 in range(4):                     # 2-bank quarters
                pw = ps.tile([128, QTR], f32, name=f"pw_{t}{g}{q4}",
                             tag="ps2", bufs=4)
                for j in range(2):                  # k-chunk pairs
                    lhsT = w8[t][:, 2 * j:2 * j + 2, g * 128:(g + 1) * 128]
                    for nb in range(2):
                        nc.tensor.matmul(
                            pw[:, nb * NBK:(nb + 1) * NBK], lhsT=lhsT,
                            rhs=x8[:, 2 * j:2 * j + 2,
                                   (2 * q4 + nb) * NBK:(2 * q4 + nb + 1) * NBK],
                            start=(j == 0), stop=(j == 1),
                            perf_mode=PM.DoubleRow)
                # rows 16*q4+1 .. 16*q4+16 of the guarded tile, contiguous
                nc.scalar.activation(
                    out=qg[:, 1 + 64 * (16 * q4 + 1):1 + 64 * (16 * q4 + 17)],
                    in_=pw, func=AF.Copy, scale=1.0 / WSCALE)
            acc = sb.tile([128, HW], bf16, name=f"acc_{t}{g}", tag="acc",
                          bufs=2)
            ssp = sb.tile([128, 4], f32, name=f"ssp_{t}{g}", tag="nrm_ssp",
                          bufs=2)
            for q4 in range(4):
                dwp = ps.tile([128, QTR], f32, name=f"dw_{t}{g}{q4}",
                              tag="ps2", bufs=4)
                for i, (ta, tb) in enumerate(DW_PAIRS):
                    offa = TOFF[ta] if ta is not None else TOFF[tb] - 2
                    d = TOFF[tb] - offa
                    lhsT = dg[t][:, g, i, :, :]
                    for nb in range(2):
                        base = 1 + (2 * q4 + nb) * NBK + offa
                        nc.tensor.matmul(
                            dwp[:, nb * NBK:(nb + 1) * NBK], lhsT=lhsT,
                            rhs=win(qg, base, [[d, 2], [1, NBK]]),
                            start=(i == 0), stop=(i == len(DW_PAIRS) - 1),
                            perf_mode=PM.DoubleRow)
                if t == "q":
                    nc.vector.tensor_copy(acc[:, q4 * QTR:(q4 + 1) * QTR],
                                          dwp)
                else:
                    nc.scalar.copy(acc[:, q4 * QTR:(q4 + 1) * QTR], dwp)
                nc.scalar.activation(out=dwp, in_=dwp, func=AF.Square,
                                     accum_out=ssp[:, q4:q4 + 1])
            si = (0 if t == "q" else G) + g
            nc.vector.tensor_tensor(out=ssp[:, 0:1], in0=ssp[:, 0:1],
                                    in1=ssp[:, 1:2], op=AO.add)
            nc.vector.tensor_tensor(out=ssp[:, 2:3], in0=ssp[:, 2:3],
                                    in1=ssp[:, 3:4], op=AO.add)
            nc.vector.tensor_tensor(out=ssall[:, si:si + 1], in0=ssp[:, 0:1],
                                    in1=ssp[:, 2:3], op=AO.add)
            # edge-wrap fixes on cols 0/63, negated taps
            acc3 = acc.rearrange("p (h w) -> p h w", w=WW)
            for dy in range(3):
                nc.vector.scalar_tensor_tensor(
                    out=acc3[:, :, 0:1],
                    in0=win(qg, 64 * dy, [[64, 64], [1, 1]]),
                    scalar=ndw[t][:, g, 3 * dy:3 * dy + 1],
                    in1=acc3[:, :, 0:1], op0=AO.mult, op1=AO.add)
                nc.vector.scalar_tensor_tensor(
                    out=acc3[:, :, 63:64],
                    in0=win(qg, 1 + 64 * (dy + 1), [[64, 64], [1, 1]]),
                    scalar=ndw[t][:, g, 3 * dy + 2:3 * dy + 3],
                    in1=acc3[:, :, 63:64], op0=AO.mult, op1=AO.add)
            tt = sb.tile([128, HW], bf16, name=f"{t}T{g}", tag=f"{t}T",
                         bufs=2)
            dst3 = tt.rearrange("p (a c) -> p a c", c=128)
            for h in range(2):
                nc.sync.dma_start(out=dst3[:, 16 * h:16 * (h + 1), :],
                                  in_=acc[:, 2048 * h:2048 * (h + 1)],
                                  transpose=True)
            (qT if t == "q" else kT)[g] = tt

        def gram(g):
            gp = ps.tile([128, QTR], f32, name=f"gram{g}", tag="ps2", bufs=4)
            for nck in range(32):
                nc.tensor.matmul(
                    gp[:, 0:128],
                    lhsT=qT[g][:, nck * 128:(nck + 1) * 128],
                    rhs=kT[g][:, nck * 128:(nck + 1) * 128],
                    start=(nck == 0), stop=(nck == 31))
            gs = sb.tile([128, 128], f32, name=f"gsb{g}", tag="gsb", bufs=4)
            nc.scalar.copy(gs, gp[:, 0:128])
            gsb[g] = gs

        # ---- v pointwise (bf16) into bordered 66-pitch pad ---------------
        def v_pw(g):
            pad = sb.tile([128, PP, PP], bf16, name=f"vpad{g}", tag="vpadp",
                          bufs=3)
            vpad[g] = pad
            padf = pad.rearrange("p h w -> p (h w)")
            nc.gpsimd.memset(padf[:, 0:PP], 0.0)
            nc.gpsimd.memset(padf[:, (PP - 1) * PP:PP * PP], 0.0)
            nc.gpsimd.memset(pad[:, 1:PP - 1, 0:1], 0.0)
            nc.gpsimd.memset(pad[:, 1:PP - 1, PP - 1:PP], 0.0)
            for q4 in range(4):
                pw = ps.tile([128, QTR], f32, name=f"vpw{g}{q4}",
                             tag="ps2", bufs=4)
                for k in range(G):
                    lhsT = wv[:, k, g * 128:(g + 1) * 128]
                    for nb in range(2):
                        nc.tensor.matmul(
                            pw[:, nb * NBK:(nb + 1) * NBK], lhsT=lhsT,
                            rhs=xb[:, k,
                                   (2 * q4 + nb) * NBK:(2 * q4 + nb + 1) * NBK],
                            start=(k == 0), stop=(k == G - 1))
                nc.scalar.copy(
                    pad[:, 1 + q4 * 16:1 + (q4 + 1) * 16, 1:WW + 1],
                    pw.rearrange("p (h w) -> p h w", w=WW))

        vdw3 = {}

        # ---- explicit v depthwise on DVE (4x mult + 2x add chain) --------
        def v_dw_dve(g, taps, seed=4):
            pad = vpad[g]
            if g not in vdw:
                if g in (3,):
                    # alias into the dead xb slot g (xb is done after v_pw(3))
                    av = xb[:, g, :]
                    vdw[g] = av
                    vdw3[g] = AP(av.tensor, av.offset,
                                 [list(av.ap[0]), [WW, HH], [1, WW]])
                else:
                    acc = sb.tile([128, HW], bf16, name=f"vdw{g}",
                                  tag=f"vdw{g}")
                    vdw[g] = acc
                    vdw3[g] = acc.rearrange("p (h w) -> p h w", w=WW)
                acc3 = vdw3[g]
                sy, sx = seed // 3, seed % 3
                nc.vector.tensor_scalar(
                    out=acc3[:, :, :], in0=pad[:, sy:sy + HH, sx:sx + WW],
                    scalar1=dwv[:, g, seed:seed + 1], scalar2=None,
                    op0=AO.mult)
            acc3 = vdw3[g]
            tmp = sb.tile([128, HH, WW], bf16, name=f"vt{g}", tag="vtmp",
                          bufs=1)
            for tap in taps:
                dy, dx = tap // 3, tap % 3
                nc.vector.tensor_scalar(
                    out=tmp, in0=pad[:, dy:dy + HH, dx:dx + WW],
                    scalar1=dwv[:, g, tap:tap + 1], scalar2=None, op0=AO.mult)
                nc.vector.tensor_tensor(out=acc3, in0=acc3, in1=tmp,
                                        op=AO.add)

        def v_dw2_gps(g, tap):
            # tap contribution in halves: DVE scaled-mult, gpsimd add
            pad = vpad[g]
            dy, dx = tap // 3, tap % 3
            for h in range(4):
                tmp = sb.tile([128, 16, WW], bf16, name=f"v2t{tap}{h}",
                              tag="vtmp2", bufs=1)
                nc.vector.tensor_scalar(
                    out=tmp, in0=pad[:, dy + 16 * h:dy + 16 * (h + 1),
                                     dx:dx + WW],
                    scalar1=dwv[:, g, tap:tap + 1], scalar2=None, op0=AO.mult)
                a3v = vdw3[g]
                nc.gpsimd.tensor_tensor(
                    out=a3v[:, 16 * h:16 * (h + 1), :],
                    in0=a3v[:, 16 * h:16 * (h + 1), :], in1=tmp, op=AO.add)

        # batched rsqrt of all 8 norms: quake bit-hack + 2 Newton steps
        def rinv_all():
            NSS = 2 * G
            sh = sb.tile([128, NSS], i32, name="nrm_sh", tag="nrm_sh")
            nc.vector.tensor_scalar(out=sh, in0=ssall.bitcast(i32), scalar1=1,
                                    scalar2=None, op0=AO.logical_shift_right)
            y0i = sb.tile([128, NSS], i32, name="nrm_y0", tag="nrm_y0")
            eng = nc.vector
            eng.add_instruction(mybir.InstTensorScalarPtr(
                name=nc.get_next_instruction_name(),
                op0=AO.subtract, reverse0=True,
                ins=[eng.lower_ap(sh[:, :]),
                     mybir.ImmediateValue(dtype=i32, value=0x5f3759df)],
                outs=[eng.lower_ap(y0i[:, :])]))
            rinv = sb.tile([128, NSS], f32, name="rinv", tag="rinv")
            nc.vector.tensor_copy(rinv, y0i.bitcast(f32))
            tn = sb.tile([128, NSS], f32, name="nrm_tn", tag="nrm_tn")
            for _ in range(2):
                nc.vector.tensor_tensor(out=tn, in0=rinv, in1=rinv, op=AO.mult)
                nc.vector.tensor_tensor(out=tn, in0=tn, in1=ssall, op=AO.mult)
                nc.vector.tensor_scalar(out=tn, in0=tn, scalar1=-0.5,
                                        scalar2=1.5, op0=AO.mult, op1=AO.add)
                nc.vector.tensor_tensor(out=rinv, in0=rinv, in1=tn, op=AO.mult)
            ts_scale = sb.tile([128, G], f32, name="tss", tag="tss")
            nc.vector.tensor_tensor(out=ts_scale, in0=tsc,
                                    in1=rinv[:, 0:G], op=AO.mult)
            return ts_scale, rinv

        # ---- attention for one chunk (2 heads), fused into proj lhsT -----
        def attn_chunk(g, ts_scale, rinv, wp):
            # logits = gram * (T*rq[d]) -> transpose -> exp(rk[e] * .)
            tmp = sb.tile([128, 128], bf16, name=f"tmp{g}", tag="atmp",
                          bufs=2)
            nc.vector.tensor_scalar(out=tmp, in0=gsb[g],
                                    scalar1=ts_scale[:, g:g + 1], scalar2=None,
                                    op0=AO.mult)
            ap_ = ps.tile([128, QTR], f32, name=f"attn{g}", tag="ps2", bufs=4)
            nc.tensor.matmul(ap_[:, 0:128], lhsT=tmp, rhs=ident,
                             start=True, stop=True)
            aexpT = sb.tile([128, 128], bf16, name=f"aexpT{g}", tag="aexpT",
                            bufs=2)
            nc.vector.memset(aexpT, 0.0)
            for blk in (0, 64):
                nc.scalar.activation(
                    out=aexpT[blk:blk + 64, blk:blk + 64],
                    in_=ap_[blk:blk + 64, blk:blk + 64],
                    func=AF.Exp, scale=rinv[blk:blk + 64, G + g:G + g + 1])
            nc.tensor.matmul(ap_[:, 512:513], lhsT=aexpT, rhs=ones,
                             start=True, stop=True)
            rs = sb.tile([128, 1], f32, name=f"rs{g}", tag="rsum", bufs=2)
            nc.vector.reciprocal(rs, ap_[:, 512:513])
            if g in FUSED:
                # fused attn@v: out = sum_tap (aexpT . dwv_tap) @ shifted v
                og = xb[:, g, :]
                a3 = []
                for tap in range(9):
                    a3t = sb.tile([128, 128], bf16, name=f"a3_{g}{tap}",
                                  tag=f"a3_{tap}", bufs=1)
                    nc.vector.tensor_scalar(out=a3t, in0=aexpT,
                                            scalar1=dwv[:, g, tap:tap + 1],
                                            scalar2=None, op0=AO.mult)
                    a3.append(a3t)
                for q4 in range(4):
                    vo = ps.tile([128, QTR], f32, name=f"fo{g}{q4}",
                                 tag="ps2", bufs=4)
                    for tap in range(9):
                        dy, dx = tap // 3, tap % 3
                        for nb in range(2):
                            r0 = (2 * q4 + nb) * 8
                            nc.tensor.matmul(
                                vo[:, nb * NBK:(nb + 1) * NBK], lhsT=a3[tap],
                                rhs=vpad[g][:, dy + r0:dy + r0 + 8,
                                            dx:dx + WW],
                                start=(tap == 0), stop=(tap == 8))
                    nc.scalar.activation(out=og[:, q4 * QTR:(q4 + 1) * QTR],
                                         in_=vo, func=AF.Copy, scale=rs)
                vdw[g] = og
                return
            # transpose back; fold rs into the evacuation scale
            nc.tensor.matmul(ap_[:, 256:384], lhsT=aexpT, rhs=ident,
                             start=True, stop=True)
            aexp = sb.tile([128, 128], bf16, name=f"aexp{g}", tag="aexp",
                           bufs=2)
            nc.scalar.activation(out=aexp, in_=ap_[:, 256:384], func=AF.Copy,
                                 scale=rs)
            # WpA_g^T = (rs*A_g)^T @ Wp_g  [e, o] - one N=512 matmul
            nc.tensor.matmul(ap_[:, 512:1024], lhsT=aexp, rhs=wp[:, g, :],
                             start=True, stop=True)
            wa = sb.tile([128, C], bf16, name=f"wpa{g}", tag="wpa", bufs=4)
            nc.scalar.copy(wa, ap_[:, 512:1024])
            wpa[g] = wa

        # ================= main schedule =================================
        kw_idx = [0]

        def keepwarm(n=4):
            kw_idx[0] += 1
            kw_ps = ps.tile([128, QTR], f32, name=f"kwps{kw_idx[0]}",
                            tag="ps2", bufs=4)
            for i in range(n):
                nc.tensor.matmul(kw_ps[:, 0:128], lhsT=warm, rhs=warm,
                                 start=(i == 0), stop=(i == n - 1))

        qk_chunk("q", 0)
        qk_chunk("k", 0)
        v_pw(0)
        gram(0)
        qk_chunk("q", 1)
        v_dw_dve(0, (0, 1, 2))
        qk_chunk("k", 1)
        v_dw_dve(0, (3, 5))
        v_pw(1)
        gram(1)
        v_dw_dve(0, (6,))
        v_dw2_gps(0, 7)
        qk_chunk("q", 2)
        v_dw_dve(0, (8,))
        qk_chunk("k", 2)
        v_pw(2)
        gram(2)
        v_dw_dve(2, (0, 1, 2))
        qk_chunk("q", 3)
        v_dw_dve(2, (3, 5))
        v_dw2_gps(2, 7)
        qk_chunk("k", 3)
        v_pw(3)
        gram(3)
        v_dw_dve(2, (6, 8))
        # wv is dead now: load proj weights into its slot
        wp = sb.tile([128, G, C], bf16, name="wp", tag="wvp")
        nc.sync.dma_start(out=wp, in_=wp_ext[:, :, :])
        v_dw_dve(3, (0, 1, 2))
        ts_scale, rinv = rinv_all()
        attn_chunk(0, ts_scale, rinv, wp)
        v_dw_dve(3, (3, 5))
        keepwarm()
        attn_chunk(2, ts_scale, rinv, wp)
        v_dw_dve(3, (7, 6))
        keepwarm()
        attn_chunk(3, ts_scale, rinv, wp)
        v_dw_dve(3, (8,))
        attn_chunk(1, ts_scale, rinv, wp)

        # ================= fused projection + store ======================
        # y[:, strip] = sum_g WpA_g^T^T @ vdw_g[:, strip]
        for q4 in range(4):
            for m in range(G):
                yp = ps.tile([128, QTR], f32, name=f"yp{m}{q4}",
                             tag="ps2", bufs=4)
                for gi, g in enumerate((0, 2, 3, 1)):
                    lhsT = (wp[:, g, m * 128:(m + 1) * 128] if g in FUSED
                            else wpa[g][:, m * 128:(m + 1) * 128])
                    for nb in range(2):
                        nc.tensor.matmul(
                            yp[:, nb * NBK:(nb + 1) * NBK], lhsT=lhsT,
                            rhs=vdw[g][:, (2 * q4 + nb) * NBK:
                                       (2 * q4 + nb + 1) * NBK],
                            start=(gi == 0), stop=(gi == G - 1))
                yt = sb.tile([128, QTR], bf16, name=f"yt{m}{q4}",
                             tag="ysb", bufs=2)
                nc.scalar.copy(yt, yp)
                nc.sync.dma_start(
                    out=out_ext[m * 128:(m + 1) * 128,
                                q4 * QTR:(q4 + 1) * QTR],
                    in_=yt)

    nc.compile()
    return nc


def prep_inputs(x, w_q, w_k, w_v, dw_q, dw_k, dw_v, w_proj, temperature):
    bf = ml_dtypes.bfloat16
    f8 = ml_dtypes.float8_e4m3
    xf = np.ascontiguousarray(np.asarray(x, np.float32)).reshape(B, C, HW)
    # [C, HW] -> [128, G, HW]
    xg = xf.reshape(B, G, 128, HW).transpose(0, 2, 1, 3)

    def wprep(w, dtype, scale=1.0):
        # w [O, I] -> lhsT layout [128 (i in chunk), G (i chunk), O]
        wt = (np.asarray(w, np.float32).T * scale).reshape(G, 128, C)
        return np.ascontiguousarray(wt.transpose(1, 0, 2)).astype(dtype)

    def dwprep(dw):
        # [C,1,3,3] -> [128, G, 9]
        d = np.asarray(dw, np.float32).reshape(G, 128, 9)
        return np.ascontiguousarray(d.transpose(1, 0, 2))

    def dgprep(dw9):
        # dw9 [128, G, 9] f32 (already fp8-rounded) -> diag pairs
        dgt = np.zeros((128, G, 5, 2, 128), np.float32)
        r = np.arange(128)
        for i, (ta, tb) in enumerate(DW_PAIRS):
            if ta is not None:
                dgt[r, :, i, 0, r] = dw9[r, :, ta]
            dgt[r, :, i, 1, r] = dw9[r, :, tb]
        return dgt.astype(f8)

    dwq9 = dwprep(dw_q).astype(f8).astype(np.float32)
    dwk9 = dwprep(dw_k).astype(f8).astype(np.float32)
    base = {
        "w8q": wprep(w_q, f8, WSCALE),
        "w8k": wprep(w_k, f8, WSCALE),
        "wv": wprep(w_v, bf),
        "wp": wprep(w_proj, bf),
        "dgq": dgprep(dwq9),
        "dgk": dgprep(dwk9),
        "ndwq": np.ascontiguousarray(-dwq9),
        "ndwk": np.ascontiguousarray(-dwk9),
        "dwv": dwprep(dw_v),
        "tsc": np.ascontiguousarray(np.repeat(
            np.asarray(temperature, np.float32).reshape(HEADS), D)
            .reshape(G, 128).T),
        "ident": np.eye(128, dtype=bf),
    }
    in_maps = []
    for b in range(B):
        m = dict(base)
        m["x8"] = np.ascontiguousarray(xg[b]).astype(f8)
        m["xb"] = np.ascontiguousarray(xg[b]).astype(bf)
        in_maps.append(m)
    return in_maps


def run(trace=False, **inputs):
    from concourse.bass_utils import run_bass_kernel_spmd

    if "nc" not in _CACHE:
        _CACHE["nc"] = _build()
    nc = _CACHE["nc"]
    in_maps = prep_inputs(**inputs)
    res = run_bass_kernel_spmd(nc, in_maps, core_ids=list(range(B)),
                               trace=trace)
    out = np.stack([np.asarray(res.results[b]["out"], np.float32)
                    for b in range(B)])
    return out.reshape(B, C, HH, WW), res


def kernel(**inputs):
    out, _ = run(trace=False, **inputs)
    return out


# revision 37
# speedup vs baseline: 1.0496x; 1.0261x over previous
"""Trainium2 Bass kernel v5.3 for channel attention (XCA-style), 8 NeuronCores.

Data-parallel over batch B=8 (1 item/core, no collectives).

Changes vs v3 (316us baseline -> ~241us):
- Post-gram normalization: q/k are transposed RAW (no khat scaling pass);
  the gram G = Q K^T is computed unnormalized and the norms are applied at
  exp time - the q row-scale (x temperature) folds into a pre-transpose
  DVE scale of the logits, the k column-scale becomes a per-partition ACT
  scale after a PE transpose. Takes rinv off the k->gram critical path.
- Softmax row-sums via a ones-column matmul on the transposed exp; rs folds
  into the transpose-back evacuation scale.
- attn@v+proj fusion for chunks 0,2,3: WpA_g^T = (rs*A_g)^T Wp_g via one
  N=512 matmul per chunk, then y = sum_g WpA_g^T^T vdw_g runs directly
  against the DVE-depthwise'd v. Chunk 1 keeps the fused 9-tap attn@v on
  the PE (fills the tail with PE work; its og aliases the dead xb slot 1,
  and the projection contracts chunks in order 0,2,1,3).
- Both q and k norm squares from the depthwise PSUM; one batched [128,8]
  DVE quake-rsqrt chain for all norms.
- q depthwise evacuation on the DVE (2x_1P PSUM read), k on ACT (split
  load); edge-wrap corrections merged to 64-row ops (6/chunk).
- keepwarm matmuls between tail attention chunks against HAM re-throttle.

Measured do-not-regress facts (all alternatives tried and worse):
- Phase 1 is 95.5% PE-busy (131.9us busy / 138.2us span) - optimal.
- Nothing on the q/k->gram critical path may queue behind multi-us
  v-depthwise taps in the DVE FIFO; in-phase-1 attention chains (+45/+70us),
  pw/dw interleave (+40), finer tap slices (+8), fused-attn1-first tail
  (+8), dense keepwarms (+5), hoisting v_pw(3)+c3 taps into phase 1 (+8)
  all regressed. The tail is cross-engine chain-latency-bound.

Kept from v3: fp8-e4m3 q/k path with DoubleRow everywhere, pitch-64 guarded
depthwise layout, fp8 weights prescaled x16, DVE v-depthwise with GpSimd
assist, PSUM in [128,1024] 2-bank groups.
"""

import os
import sys

import numpy as np

for _p in ("/opt/trn_rl_repo", "/root/.axon_site/_ro/trn_rl_repo"):
    if os.path.isdir(_p) and _p not in sys.path:
        sys.path.insert(0, _p)

import ml_dtypes

B, C, HH, WW = 8, 512, 64, 64
HEADS, D = 8, 64
HW = HH * WW          # 4096
G = C // 128          # 4 chunks of 128 channels (2 heads each)
NBK = 512             # one PSUM bank of fp32
PP = WW + 2           # bordered pad pitch for v (66)
NG = 66 * 64 + 2      # pitch-64 guarded flat size for q/k (4226)
WSCALE = 16.0         # fp8 weight prescale (undone at pw evacuation)
QTR = 1024            # 2-bank PSUM group width

# tap index t = dy*3+dx; flat stream offset for bank nb: 1 + nb*512 + TOFF[t]
TOFF = [dy * 64 + dx - 1 for dy in range(3) for dx in range(3)]
# DoubleRow tap pairs (a, b): dim1 stride must be >= 2 (d=1 wedges the device)
DW_PAIRS = [(0, 2), (1, 4), (3, 6), (5, 8), (None, 7)]
FUSED = (1,)

_CACHE = {}


def _build():
    from contextlib import ExitStack

    import concourse.tile as tile
    from concourse import bacc, mybir
    from concourse.ap import AP

    f32 = mybir.dt.float32
    bf16 = mybir.dt.bfloat16
    f8 = mybir.dt.float8e4
    i32 = mybir.dt.int32
    AO = mybir.AluOpType
    AF = mybir.ActivationFunctionType
    PM = mybir.MatmulPerfMode

    nc = bacc.Bacc()

    x8_ext = nc.declare_dram_parameter("x8", [128, G, HW], f8, isOutput=False)
    xb_ext = nc.declare_dram_parameter("xb", [128, G, HW], bf16, isOutput=False)
    w8_ext = {t: nc.declare_dram_parameter(f"w8{t}", [128, G, C], f8,
                                           isOutput=False) for t in "qk"}
    wv_ext = nc.declare_dram_parameter("wv", [128, G, C], bf16, isOutput=False)
    wp_ext = nc.declare_dram_parameter("wp", [128, G, C], bf16, isOutput=False)
    dg_ext = {t: nc.declare_dram_parameter(f"dg{t}", [128, G, 5, 2, 128], f8,
                                           isOutput=False) for t in "qk"}
    ndw_ext = {t: nc.declare_dram_parameter(f"ndw{t}", [128, G, 9], f32,
                                            isOutput=False) for t in "qk"}
    dwv_ext = nc.declare_dram_parameter("dwv", [128, G, 9], f32, isOutput=False)
    tsc_ext = nc.declare_dram_parameter("tsc", [128, G], f32, isOutput=False)
    id_ext = nc.declare_dram_parameter("ident", [128, 128], bf16, isOutput=False)
    out_ext = nc.declare_dram_parameter("out", [C, HW], bf16, isOutput=True)

    with ExitStack() as ctx:
        tc = ctx.enter_context(tile.TileContext(nc))
        sb = ctx.enter_context(tc.tile_pool(name="sb", bufs=1))
        ps = ctx.enter_context(tc.tile_pool(name="ps", bufs=1, space="PSUM"))

        def win(tile_, off, dims):
            a = tile_[:, off:off + 1]
            return AP(a.tensor, a.offset, [list(a.ap[0])] + dims)

        # ---- PE warm-up: ramp the pstate during the initial DMA wait ----
        warm = sb.tile([128, 128], bf16, name="warm", tag="warm")
        nc.vector.memset(warm, 0.0)
        ones = sb.tile([128, 1], bf16, name="ones", tag="ones")
        nc.vector.memset(ones, 1.0)
        wp_ps = ps.tile([128, NBK], f32, name="warmps", tag="ps2", bufs=4)
        for i in range(56):
            nc.tensor.matmul(wp_ps[:, 0:128], lhsT=warm, rhs=warm,
                             start=(i == 0), stop=(i == 55))

        # ---- persistent loads (q-path first so pw can start ASAP) -------
        w8q = sb.tile([128, G, C], f8, name="w8q", tag="w8q")
        nc.sync.dma_start(out=w8q, in_=w8_ext["q"][:, :, :])
        x8 = sb.tile([128, G, HW], f8, name="x8", tag="x8")
        nc.sync.dma_start(out=x8[:, 0:2, :], in_=x8_ext[:, 0:2, :])
        nc.sync.dma_start(out=x8[:, 2:4, :], in_=x8_ext[:, 2:4, :])
        dgq = sb.tile([128, G, 5, 2, 128], f8, name="dgq", tag="dgq")
        nc.sync.dma_start(out=dgq, in_=dg_ext["q"][:, :, :, :, :])
        ndwq = sb.tile([128, G, 9], f32, name="ndwq", tag="ndwq")
        nc.sync.dma_start(out=ndwq, in_=ndw_ext["q"][:, :, :])
        tsc = sb.tile([128, G], f32, name="tsc", tag="tsc")
        nc.sync.dma_start(out=tsc, in_=tsc_ext[:, :])
        w8k = sb.tile([128, G, C], f8, name="w8k", tag="w8k")
        nc.sync.dma_start(out=w8k, in_=w8_ext["k"][:, :, :])
        dgk = sb.tile([128, G, 5, 2, 128], f8, name="dgk", tag="dgk")
        nc.sync.dma_start(out=dgk, in_=dg_ext["k"][:, :, :, :, :])
        ndwk = sb.tile([128, G, 9], f32, name="ndwk", tag="ndwk")
        nc.sync.dma_start(out=ndwk, in_=ndw_ext["k"][:, :, :])
        xb = sb.tile([128, G, HW], bf16, name="xb", tag="xb")
        nc.sync.dma_start(out=xb, in_=xb_ext[:, :, :])
        wv = sb.tile([128, G, C], bf16, name="wv", tag="wvp")
        nc.sync.dma_start(out=wv, in_=wv_ext[:, :, :])
        dwv = sb.tile([128, G, 9], f32, name="dwv", tag="dwv")
        nc.sync.dma_start(out=dwv, in_=dwv_ext[:, :, :])
        ident = sb.tile([128, 128], bf16, name="ident", tag="ident")
        nc.sync.dma_start(out=ident, in_=id_ext[:, :])

        w8 = {"q": w8q, "k": w8k}
        dg = {"q": dgq, "k": dgk}
        ndw = {"q": ndwq, "k": ndwk}

        # persistent per-chunk results
        vpad = [None] * G
        qT = [None] * G
        kT = [None] * G
        vdw = {}
        wpa = [None] * G
        gsb = [None] * G
        ssall = sb.tile([128, 2 * G], f32, name="ssall", tag="ssall")

        # ---- q/k pointwise + depthwise, all fp8 DoubleRow ----------------
        def qk_chunk(t, g):
            """fp8 pw into guarded qg, fp8-DR depthwise into bf16 acc,
            PSUM norm squares, corrections, raw transpose."""
            qg = sb.tile([128, NG], f8, name=f"qg_{t}{g}", tag="qg", bufs=2)
            # zero guards: front elem + row0 [0..64], row65 + tail [4161..]
            nc.gpsimd.memset(qg[:, 0:65], 0.0)
            nc.gpsimd.memset(qg[:, 1 + 64 * 65:NG], 0.0)
            for q4 in range(4):                     # 2-bank quarters
                pw = ps.tile([128, QTR], f32, name=f"pw_{t}{g}{q4}",
                             tag="ps2", bufs=4)
                for j in range(2):                  # k-chunk pairs
                    lhsT = w8[t][:, 2 * j:2 * j + 2, g * 128:(g + 1) * 128]
                    for nb in range(2):
                        nc.tensor.matmul(
                            pw[:, nb * NBK:(nb + 1) * NBK], lhsT=lhsT,
                            rhs=x8[:, 2 * j:2 * j + 2,
                                   (2 * q4 + nb) * NBK:(2 * q4 + nb + 1) * NBK],
                            start=(j == 0), stop=(j == 1),
                            perf_mode=PM.DoubleRow)
                # rows 16*q4+1 .. 16*q4+16 of the guarded tile, contiguous
                nc.scalar.activation(
                    out=qg[:, 1 + 64 * (16 * q4 + 1):1 + 64 * (16 * q4 + 17)],
                    in_=pw, func=AF.Copy, scale=1.0 / WSCALE)
            acc = sb.tile([128, HW], bf16, name=f"acc_{t}{g}", tag="acc",
                          bufs=2)
            ssp = sb.tile([128, 4], f32, name=f"ssp_{t}{g}", tag="nrm_ssp",
                          bufs=2)
            for q4 in range(4):
                dwp = ps.tile([128, QTR], f32, name=f"dw_{t}{g}{q4}",
                              tag="ps2", bufs=4)
                for i, (ta, tb) in enumerate(DW_PAIRS):
                    offa = TOFF[ta] if ta is not None else TOFF[tb] - 2
                    d = TOFF[tb] - offa
                    lhsT = dg[t][:, g, i, :, :]
                    for nb in range(2):
                        base = 1 + (2 * q4 + nb) * NBK + offa
                        nc.tensor.matmul(
                            dwp[:, nb * NBK:(nb + 1) * NBK], lhsT=lhsT,
                            rhs=win(qg, base, [[d, 2], [1, NBK]]),
                            start=(i == 0), stop=(i == len(DW_PAIRS) - 1),
                            perf_mode=PM.DoubleRow)
                if t == "q":
                    nc.vector.tensor_copy(acc[:, q4 * QTR:(q4 + 1) * QTR],
                                          dwp)
                else:
                    nc.scalar.copy(acc[:, q4 * QTR:(q4 + 1) * QTR], dwp)
                nc.scalar.activation(out=dwp, in_=dwp, func=AF.Square,
                                     accum_out=ssp[:, q4:q4 + 1])
            si = (0 if t == "q" else G) + g
            nc.vector.tensor_tensor(out=ssp[:, 0:1], in0=ssp[:, 0:1],
                                    in1=ssp[:, 1:2], op=AO.add)
            nc.vector.tensor_tensor(out=ssp[:, 2:3], in0=ssp[:, 2:3],
                                    in1=ssp[:, 3:4], op=AO.add)
            nc.vector.tensor_tensor(out=ssall[:, si:si + 1], in0=ssp[:, 0:1],
                                    in1=ssp[:, 2:3], op=AO.add)
            # edge-wrap fixes on cols 0/63, negated taps
            acc3 = acc.rearrange("p (h w) -> p h w", w=WW)
            for dy in range(3):
                nc.vector.scalar_tensor_tensor(
                    out=acc3[:, :, 0:1],
                    in0=win(qg, 64 * dy, [[64, 64], [1, 1]]),
                    scalar=ndw[t][:, g, 3 * dy:3 * dy + 1],
                    in1=acc3[:, :, 0:1], op0=AO.mult, op1=AO.add)
                nc.vector.scalar_tensor_tensor(
                    out=acc3[:, :, 63:64],
                    in0=win(qg, 1 + 64 * (dy + 1), [[64, 64], [1, 1]]),
                    scalar=ndw[t][:, g, 3 * dy + 2:3 * dy + 3],
                    in1=acc3[:, :, 63:64], op0=AO.mult, op1=AO.add)
            tt = sb.tile([128, HW], bf16, name=f"{t}T{g}", tag=f"{t}T",
                         bufs=2)
            dst3 = tt.rearrange("p (a c) -> p a c", c=128)
            for h in range(2):
                nc.sync.dma_start(out=dst3[:, 16 * h:16 * (h + 1), :],
                                  in_=acc[:, 2048 * h:2048 * (h + 1)],
                                  transpose=True)
            (qT if t == "q" else kT)[g] = tt

        def gram(g):
            gp = ps.tile([128, QTR], f32, name=f"gram{g}", tag="ps2", bufs=4)
            for nck in range(32):
                nc.tensor.matmul(
                    gp[:, 0:128],
                    lhsT=qT[g][:, nck * 128:(nck + 1) * 128],
                    rhs=kT[g][:, nck * 128:(nck + 1) * 128],
                    start=(nck == 0), stop=(nck == 31))
            gs = sb.tile([128, 128], f32, name=f"gsb{g}", tag="gsb", bufs=4)
            nc.scalar.copy(gs, gp[:, 0:128])
            gsb[g] = gs

        # ---- v pointwise (bf16) into bordered 66-pitch pad ---------------
        def v_pw(g):
            pad = sb.tile([128, PP, PP], bf16, name=f"vpad{g}", tag="vpadp",
                          bufs=3)
            vpad[g] = pad
            padf = pad.rearrange("p h w -> p (h w)")
            nc.gpsimd.memset(padf[:, 0:PP], 0.0)
            nc.gpsimd.memset(padf[:, (PP - 1) * PP:PP * PP], 0.0)
            nc.gpsimd.memset(pad[:, 1:PP - 1, 0:1], 0.0)
            nc.gpsimd.memset(pad[:, 1:PP - 1, PP - 1:PP], 0.0)
            for q4 in range(4):
                pw = ps.tile([128, QTR], f32, name=f"vpw{g}{q4}",
                             tag="ps2", bufs=4)
                for k in range(G):
                    lhsT = wv[:, k, g * 128:(g + 1) * 128]
                    for nb in range(2):
                        nc.tensor.matmul(
                            pw[:, nb * NBK:(nb + 1) * NBK], lhsT=lhsT,
                            rhs=xb[:, k,
                                   (2 * q4 + nb) * NBK:(2 * q4 + nb + 1) * NBK],
                            start=(k == 0), stop=(k == G - 1))
                nc.scalar.copy(
                    pad[:, 1 + q4 * 16:1 + (q4 + 1) * 16, 1:WW + 1],
                    pw.rearrange("p (h w) -> p h w", w=WW))

        vdw3 = {}

        # ---- explicit v depthwise on DVE (4x mult + 2x add chain) --------
        def v_dw_dve(g, taps, seed=4):
            pad = vpad[g]
            if g not in vdw:
                if g in (3,):
                    # alias into the dead xb slot g (xb is done after v_pw(3))
                    av = xb[:, g, :]
                    vdw[g] = av
                    vdw3[g] = AP(av.tensor, av.offset,
                                 [list(av.ap[0]), [WW, HH], [1, WW]])
                else:
                    acc = sb.tile([128, HW], bf16, name=f"vdw{g}",
                                  tag=f"vdw{g}")
                    vdw[g] = acc
                    vdw3[g] = acc.rearrange("p (h w) -> p h w", w=WW)
                acc3 = vdw3[g]
                sy, sx = seed // 3, seed % 3
                nc.vector.tensor_scalar(
                    out=acc3[:, :, :], in0=pad[:, sy:sy + HH, sx:sx + WW],
                    scalar1=dwv[:, g, seed:seed + 1], scalar2=None,
                    op0=AO.mult)
            acc3 = vdw3[g]
            tmp = sb.tile([128, HH, WW], bf16, name=f"vt{g}", tag="vtmp",
                          bufs=1)
            for tap in taps:
                dy, dx = tap // 3, tap % 3
                nc.vector.tensor_scalar(
                    out=tmp, in0=pad[:, dy:dy + HH, dx:dx + WW],
                    scalar1=dwv[:, g, tap:tap + 1], scalar2=None, op0=AO.mult)
                nc.vector.tensor_tensor(out=acc3, in0=acc3, in1=tmp,
                                        op=AO.add)

        def v_dw2_gps(g, tap):
            # tap contribution in halves: DVE scaled-mult, gpsimd add
            pad = vpad[g]
            dy, dx = tap // 3, tap % 3
            for h in range(4):
                tmp = sb.tile([128, 16, WW], bf16, name=f"v2t{tap}{h}",
                              tag="vtmp2", bufs=1)
                nc.vector.tensor_scalar(
                    out=tmp, in0=pad[:, dy + 16 * h:dy + 16 * (h + 1),
                                     dx:dx + WW],
                    scalar1=dwv[:, g, tap:tap + 1], scalar2=None, op0=AO.mult)
                a3v = vdw3[g]
                nc.gpsimd.tensor_tensor(
                    out=a3v[:, 16 * h:16 * (h + 1), :],
                    in0=a3v[:, 16 * h:16 * (h + 1), :], in1=tmp, op=AO.add)

        # batched rsqrt of all 8 norms: quake bit-hack + 2 Newton steps
        def rinv_all():
            NSS = 2 * G
            sh = sb.tile([128, NSS], i32, name="nrm_sh", tag="nrm_sh")
            nc.vector.tensor_scalar(out=sh, in0=ssall.bitcast(i32), scalar1=1,
                                    scalar2=None, op0=AO.logical_shift_right)
            y0i = sb.tile([128, NSS], i32, name="nrm_y0", tag="nrm_y0")
            eng = nc.vector
            eng.add_instruction(mybir.InstTensorScalarPtr(
                name=nc.get_next_instruction_name(),
                op0=AO.subtract, reverse0=True,
                ins=[eng.lower_ap(sh[:, :]),
                     mybir.ImmediateValue(dtype=i32, value=0x5f3759df)],
                outs=[eng.lower_ap(y0i[:, :])]))
            rinv = sb.tile([128, NSS], f32, name="rinv", tag="rinv")
            nc.vector.tensor_copy(rinv, y0i.bitcast(f32))
            tn = sb.tile([128, NSS], f32, name="nrm_tn", tag="nrm_tn")
            for _ in range(2):
                nc.vector.tensor_tensor(out=tn, in0=rinv, in1=rinv, op=AO.mult)
                nc.vector.tensor_tensor(out=tn, in0=tn, in1=ssall, op=AO.mult)
                nc.vector.tensor_scalar(out=tn, in0=tn, scalar1=-0.5,
                                        scalar2=1.5, op0=AO.mult, op1=AO.add)
                nc.vector.tensor_tensor(out=rinv, in0=rinv, in1=tn, op=AO.mult)
            ts_scale = sb.tile([128, G], f32, name="tss", tag="tss")
            nc.vector.tensor_tensor(out=ts_scale, in0=tsc,
                                    in1=rinv[:, 0:G], op=AO.mult)
            return ts_scale, rinv

        # ---- attention for one chunk (2 heads), fused into proj lhsT -----
        def attn_chunk(g, ts_scale, rinv, wp):
            # logits = gram * (T*rq[d]) -> transpose -> exp(rk[e] * .)
            tmp = sb.tile([128, 128], bf16, name=f"tmp{g}", tag="atmp",
                          bufs=2)
            nc.vector.tensor_scalar(out=tmp, in0=gsb[g],
                                    scalar1=ts_scale[:, g:g + 1], scalar2=None,
                                    op0=AO.mult)
            ap_ = ps.tile([128, QTR], f32, name=f"attn{g}", tag="ps2", bufs=4)
            nc.tensor.matmul(ap_[:, 0:128], lhsT=tmp, rhs=ident,
                             start=True, stop=True)
            aexpT = sb.tile([128, 128], bf16, name=f"aexpT{g}", tag="aexpT",
                            bufs=2)
            nc.vector.memset(aexpT, 0.0)
            for blk in (0, 64):
                nc.scalar.activation(
                    out=aexpT[blk:blk + 64, blk:blk + 64],
                    in_=ap_[blk:blk + 64, blk:blk + 64],
                    func=AF.Exp, scale=rinv[blk:blk + 64, G + g:G + g + 1])
            nc.tensor.matmul(ap_[:, 512:513], lhsT=aexpT, rhs=ones,
                             start=True, stop=True)
            rs = sb.tile([128, 1], f32, name=f"rs{g}", tag="rsum", bufs=2)
            nc.vector.reciprocal(rs, ap_[:, 512:513])
            if g in FUSED:
                # fused attn@v: out = sum_tap (aexpT . dwv_tap) @ shifted v
                og = xb[:, g, :]
                a3 = []
                for tap in range(9):
                    a3t = sb.tile([128, 128], bf16, name=f"a3_{g}{tap}",
                                  tag=f"a3_{tap}", bufs=1)
                    nc.vector.tensor_scalar(out=a3t, in0=aexpT,
                                            scalar1=dwv[:, g, tap:tap + 1],
                                            scalar2=None, op0=AO.mult)
                    a3.append(a3t)
                for q4 in range(4):
                    vo = ps.tile([128, QTR], f32, name=f"fo{g}{q4}",
                                 tag="ps2", bufs=4)
                    for tap in range(9):
                        dy, dx = tap // 3, tap % 3
                        for nb in range(2):
                            r0 = (2 * q4 + nb) * 8
                            nc.tensor.matmul(
                                vo[:, nb * NBK:(nb + 1) * NBK], lhsT=a3[tap],
                                rhs=vpad[g][:, dy + r0:dy + r0 + 8,
                                            dx:dx + WW],
                                start=(tap == 0), stop=(tap == 8))
                    nc.scalar.activation(out=og[:, q4 * QTR:(q4 + 1) * QTR],
                                         in_=vo, func=AF.Copy, scale=rs)
                vdw[g] = og
                return
            # transpose back; fold rs into the evacuation scale
            nc.tensor.matmul(ap_[:, 256:384], lhsT=aexpT, rhs=ident,
                             start=True, stop=True)
            aexp = sb.tile([128, 128], bf16, name=f"aexp{g}", tag="aexp",
                           bufs=2)
            nc.scalar.activation(out=aexp, in_=ap_[:, 256:384], func=AF.Copy,
                                 scale=rs)
            # WpA_g^T = (rs*A_g)^T @ Wp_g  [e, o] - one N=512 matmul
            nc.tensor.matmul(ap_[:, 512:1024], lhsT=aexp, rhs=wp[:, g, :],
                             start=True, stop=True)
            wa = sb.tile([128, C], bf16, name=f"wpa{g}", tag="wpa", bufs=4)
            nc.scalar.copy(wa, ap_[:, 512:1024])
            wpa[g] = wa

        # ================= main schedule =================================
        kw_idx = [0]

        def keepwarm(n=4):
            kw_idx[0] += 1
            kw_ps = ps.tile([128, QTR], f32, name=f"kwps{kw_idx[0]}",
                            tag="ps2", bufs=4)
            for i in range(n):
                nc.tensor.matmul(kw_ps[:, 0:128], lhsT=warm, rhs=warm,
                                 start=(i == 0), stop=(i == n - 1))

        qk_chunk("q", 0)
        qk_chunk("k", 0)
        v_pw(0)
        gram(0)
        qk_chunk("q", 1)
        v_dw_dve(0, (0, 1, 2))
        qk_chunk("k", 1)
        v_dw_dve(0, (3, 5))
        v_pw(1)
        gram(1)
        v_dw_dve(0, (6,))
        v_dw2_gps(0, 7)
        qk_chunk("q", 2)
        v_dw_dve(0, (8,))
        qk_chunk("k", 2)
        v_pw(2)
        gram(2)
        v_dw_dve(2, (0, 1, 2))
        qk_chunk("q", 3)
        v_dw_dve(2, (3, 5))
        v_dw2_gps(2, 7)
        qk_chunk("k", 3)
        v_pw(3)
        gram(3)
        v_dw_dve(2, (6, 8))
        # wv is dead now: load proj weights into its slot
        wp = sb.tile([128, G, C], bf16, name="wp", tag="wvp")
        nc.sync.dma_start(out=wp, in_=wp_ext[:, :, :])
        v_dw_dve(3, (0, 1, 2))
        ts_scale, rinv = rinv_all()
        attn_chunk(0, ts_scale, rinv, wp)
        v_dw_dve(3, (3, 5))
        keepwarm()
        attn_chunk(2, ts_scale, rinv, wp)
        v_dw_dve(3, (7, 6))
        keepwarm()
        attn_chunk(3, ts_scale, rinv, wp)
        v_dw_dve(3, (8,))
        attn_chunk(1, ts_scale, rinv, wp)

        # ================= fused projection + store ======================
        # y[:, strip] = sum_g WpA_g^T^T @ vdw_g[:, strip]
        for q4 in range(4):
            for m in range(G):
                yp = ps.tile([128, QTR], f32, name=f"yp{m}{q4}",
                             tag="ps2", bufs=4)
                for gi, g in enumerate((0, 2, 3, 1)):
                    lhsT = (wp[:, g, m * 128:(m + 1) * 128] if g in FUSED
                            else wpa[g][:, m * 128:(m + 1) * 128])
                    for nb in range(2):
                        nc.tensor.matmul(
                            yp[:, nb * NBK:(nb + 1) * NBK], lhsT=lhsT,
                            rhs=vdw[g][:, (2 * q4 + nb) * NBK:
                                       (2 * q4 + nb + 1) * NBK],
                            start=(gi == 0), stop=(gi == G - 1))
                yt = sb.tile([128, QTR], bf16, name=f"yt{m}{q4}",
                             tag="ysb", bufs=2)
                nc.scalar.copy(yt, yp)
                nc.sync.dma_start(
                    out=out_ext[m * 128:(m + 1) * 128,
                                q4 * QTR:(q4 + 1) * QTR],
                    in_=yt)

    nc.compile()
    return nc


def prep_inputs(x, w_q, w_k, w_v, dw_q, dw_k, dw_v, w_proj, temperature):
    bf = ml_dtypes.bfloat16
    f8 = ml_dtypes.float8_e4m3
    xf = np.ascontiguousarray(np.asarray(x, np.float32)).reshape(B, C, HW)
    # [C, HW] -> [128, G, HW]
    xg = xf.reshape(B, G, 128, HW).transpose(0, 2, 1, 3)

    def wprep(w, dtype, scale=1.0):
        # w [O, I] -> lhsT layout [128 (i in chunk), G (i chunk), O]
        wt = (np.asarray(w, np.float32).T * scale).reshape(G, 128, C)
        return np.ascontiguousarray(wt.transpose(1, 0, 2)).astype(dtype)

    def dwprep(dw):
        # [C,1,3,3] -> [128, G, 9]
        d = np.asarray(dw, np.float32).reshape(G, 128, 9)
        return np.ascontiguousarray(d.transpose(1, 0, 2))

    def dgprep(dw9):
        # dw9 [128, G, 9] f32 (already fp8-rounded) -> diag pairs
        dgt = np.zeros((128, G, 5, 2, 128), np.float32)
        r = np.arange(128)
        for i, (ta, tb) in enumerate(DW_PAIRS):
            if ta is not None:
                dgt[r, :, i, 0, r] = dw9[r, :, ta]
            dgt[r, :, i, 1, r] = dw9[r, :, tb]
        return dgt.astype(f8)

    dwq9 = dwprep(dw_q).astype(f8).astype(np.float32)
    dwk9 = dwprep(dw_k).astype(f8).astype(np.float32)
    base = {
        "w8q": wprep(w_q, f8, WSCALE),
        "w8k": wprep(w_k, f8, WSCALE),
        "wv": wprep(w_v, bf),
        "wp": wprep(w_proj, bf),
        "dgq": dgprep(dwq9),
        "dgk": dgprep(dwk9),
        "ndwq": np.ascontiguousarray(-dwq9),
        "ndwk": np.ascontiguousarray(-dwk9),
        "dwv": dwprep(dw_v),
        "tsc": np.ascontiguousarray(np.repeat(
            np.asarray(temperature, np.float32).reshape(HEADS), D)
            .reshape(G, 128).T),
        "ident": np.eye(128, dtype=bf),
    }
    in_maps = []
    for b in range(B):
        m = dict(base)
        m["x8"] = np.ascontiguousarray(xg[b]).astype(f8)
        m["xb"] = np.ascontiguousarray(xg[b]).astype(bf)
        in_maps.append(m)
    return in_maps


def run(trace=False, **inputs):
    from concourse.bass_utils import run_bass_kernel_spmd

    if "nc" not in _CACHE:
        _CACHE["nc"] = _build()
    nc = _CACHE["nc"]
    in_maps = prep_inputs(**inputs)
    res = run_bass_kernel_spmd(nc, in_maps, core_ids=list(range(B)),
                               trace=trace)
    out = np.stack([np.asarray(res.results[b]["out"], np.float32)
                    for b in range(B)])
    return out.reshape(B, C, HH, WW), res


def kernel(**inputs):
    out, _ = run(trace=False, **inputs)
    return out
